# revision 46
# baseline (speedup 1.0000x reference)
"""LIF (leaky integrate-and-fire) spiking-neuron scan on 8 Trainium2 NeuronCores.

Reference semantics (per element, f32):
    h_t = v_{t-1} + (x_t - v_{t-1}) / 2        (tau = 2, v_reset = 0)
    s_t = (h_t >= 1)                           (spike, threshold v_th = 1)
    v_t = h_t * (1 - s_t)                      (hard reset)

Device formulation: shifted pre-activation u_t = v_{t-1} + x_t - 2, so
s_t = (u_t >= 0) and the whole step is ONE fused custom-DVE op:
    u_t = (0.5 * u_{t-1} + 1) * (u_{t-1} < 0) + (x_t - 2)
(x - 2 is precomputed on host; u_0 = -2 encodes v_0 = 0.)  The custom op
(registered into concourse.dve_ops at import, compiled into the per-NEFF
DVE table) runs at 1 elem/cycle/lane fp32 — one ~691 ns instruction per
timestep, ~727 ns dependent-chain pitch, bit-exact f32 arithmetic.

Output: u_t cast fp32 -> fp8e4 by the (otherwise idle) ACT engine, then
stored via HWDGE.  Spike <=> u_t >= 0 <=> fp8 sign bit clear (sign
survives rounding and saturation), so the host decodes
spikes = (u8_bits < 0x80).  Casting on ACT instead of during the DMA
keeps the 16.8 MB of fp32 u-reads off the SDMA/SBUF-AXI budget: DMA
moves only 16.8 MB in + 4.2 MB out per core vs 46.5 us of DVE time.
Loads get a dedicated HWDGE ring (SP); casts + stores ride the ACT ring.

Sharding: batch dim B=64 split across 8 cores (8 rows each); time stays
local.  DRAM layout is partition-major [128, T*512] so every DMA segment
is contiguous per partition.
"""

import os
import numpy as np

T, B, N = 64, 64, 8192
NCORES = 8
BL = B // NCORES          # batch rows per core
P = 128                   # SBUF partitions
F = (BL * N) // P         # free elems per partition per step  (512)

# timestep chunking: small first chunks prime the pipeline, then steady-state
# (even count and symmetric sizes keep the two HWDGE rings byte-balanced)
# small first chunks matter twice over: HWDGE rings complete their ~4
# in-flight transfers round-robin at packet granularity, so a chunk's
# completion latency is ~4x its own transfer time
LOAD_CHUNKS = [1, 1, 1, 1, 2, 2] + [4] * 14
assert sum(LOAD_CHUNKS) == T
UC = 8                    # u-history chunk (timesteps per SBUF u buffer)
CAST = 4                  # timesteps per ACT Sign op
TAPER = 1                 # Sign granularity for the last TAIL steps
TAIL = 4                  # final steps signed one-by-one to shrink the tail
GL = 7                    # max timesteps packed per balanced-ternary group
# pack groups (start, len): 7-step PE-packed groups through t=55, a 4-step
# group, then single-step raw-Sign groups so the post-compute tail never
# waits on a PE round trip
GROUPS = [(7 * m, 7) for m in range(8)] + [(56, 4)] + [(60 + i, 1) for i in range(4)]
assert sum(g[1] for g in GROUPS) == T
NG = len(GROUPS)
GRP_OF = {}
for gi, (gs, gl) in enumerate(GROUPS):
    for tt in range(gs, gs + gl):
        GRP_OF[tt] = (gi, gs, gl)

_built = {}


def _register_lif_op():
    from concourse import dve_ops
    from concourse.dve_spec import (
        Spec, Src0, Src1, C0, Zero, One, lower, _has_src1,
    )
    from concourse.dve_uop import DveOpSpec

    for op in dve_ops.OPS:
        if op.name == "LIF_STEP_ANT":
            return op

    body = (Src0 * C0 + One) * (Src0 < Zero) + Src1

    def ref(in0, in1, s0, s1, imm2):
        f = np.float32
        mask = (in0 < 0).astype(f)
        return ((in0 * f(s0) + f(1.0)) * mask + in1).astype(f)

    spec = Spec(body=body, reference=ref)
    name = "LIF_STEP_ANT"
    row = dve_ops._CUSTOM_DVE_ROW_BASE + len(dve_ops.OPS)
    shas = {}
    for ver in ("v3", "v4"):
        tmp = DveOpSpec(
            name=name, opcode=row, uops=lower(spec, ver=ver),
            rd1_en=_has_src1(spec),
        )
        shas[ver] = tmp.sha(ver)
    op = dve_ops.DveOp(name, spec, subdim=False, uops_sha=shas)
    dve_ops.OPS.append(op)
    dve_ops._SUB_OPCODE_FOR_NAME[name] = row
    dve_ops.CUSTOM_DVE_SPECS[name] = spec
    return op


def _build():
    if "nc" in _built:
        return _built["nc"]

    from contextlib import ExitStack
    import concourse.mybir as mybir
    from concourse import bacc, tile

    # Slim the kernel-exit choreography: the stock exit is
    # drain -> all_engine_barrier -> clear sems -> all_engine_barrier; the
    # trailing barrier only orders the sem clears against later instructions,
    # of which there are none at kernel end (~3us saved).
    from concourse.vector_clock import ScopedClock

    def _slim_drain_and_barrier(self, tick_clock, wait_clock):
        drain_inst = self.nc.sync.drain()
        wait_clock.add_sem_waits(
            drain_inst.ins, ScopedClock({None: tick_clock.global_clock})
        )
        self.nc.all_engine_barrier()
        popped = self.nc._tile_sem_poison_stack.pop()
        assert popped is self._sem_poison
        self.nc.clear_and_free_semaphores(list(self.sems.allocated().values()))

    tile.TileContext._drain_and_barrier = _slim_drain_and_barrier

    lif_op = _register_lif_op()

    nc = bacc.Bacc("TRN2", target_bir_lowering=False, debug=False)
    # partition-major layouts: [P, T*F] so per-partition bytes are contiguous
    x_ext = nc.dram_tensor("x", [P, T * F], mybir.dt.float32, kind="ExternalInput")
    # 3^k-scaled identity weights for the balanced-ternary spike packer
    w3_ext = nc.dram_tensor("w3", [P, GL * P], mybir.dt.float16, kind="ExternalInput")
    # packed output: NG fp16 tiles, each holding sum_k 3^k * sign(u_{7m+k} )
    pk_ext = nc.dram_tensor("pk", [P, NG * F], mybir.dt.float16, kind="ExternalOutput")

    Copy = mybir.ActivationFunctionType.Copy
    Sign = mybir.ActivationFunctionType.Sign

    # scratch target for the store-gate dummy DMA
    gate_ext = nc.dram_tensor("gate", [1, 1], mybir.dt.float32, kind="Internal")

    with tile.TileContext(nc) as tc:
        with ExitStack() as ctx:
            # x chunks and fp8 output tiles are fully resident (unique tags,
            # no recycling): load issues are never gated on buffer reuse, so
            # both HWDGE rings stream the whole 16 MiB input uninterrupted.
            # x chunks recycle through 12 buffers: deep enough that a load
            # issue is never gated on far-future consumption, shallow enough
            # that the issue pacing keeps each HWDGE ring's in-flight queue
            # (~4 transfers deep) from blocking its engine for long.
            xp = ctx.enter_context(tc.tile_pool(name="xp", bufs=11))
            up = ctx.enter_context(tc.tile_pool(name="up", bufs=5))
            gp = ctx.enter_context(tc.tile_pool(name="gp", bufs=3))
            kp = ctx.enter_context(tc.tile_pool(name="kp", bufs=1))
            pp = ctx.enter_context(tc.tile_pool(name="pp", bufs=2, space="PSUM"))
            ip = ctx.enter_context(tc.tile_pool(name="ip", bufs=1))

            u0 = ip.tile([P, F], mybir.dt.float32)
            nc.vector.memset(u0[:], -2.0)
            w3 = ip.tile([P, GL * P], mybir.dt.float16)
            nc.sync.dma_start(out=w3[:], in_=w3_ext[:, :])

            # issue all loads up front, alternating the two HWDGE rings
            # (byte-balanced)
            x_tiles = []
            t0 = 0
            for i, ch in enumerate(LOAD_CHUNKS):
                xt = xp.tile([P, ch * F], mybir.dt.float32, tag="xchunk")
                dma_eng = nc.sync if i % 2 == 0 else nc.scalar
                dma_eng.dma_start(out=xt[:], in_=x_ext[:, t0 * F:(t0 + ch) * F])
                x_tiles.append((t0, ch, xt))
                t0 += ch

            # zero bias for Sign (bias 0.0 is a pre-registered const AP)
            prev = u0[:]
            uc = None
            gc = None
            pk_ps = None
            pk_tiles = []   # (group idx, SBUF fp16 pack tile)

            def emit_signs(t, n):
                # ACT: g = Sign(u) in fp16 for steps (t-n, t]; {-1,0,+1}
                # exactly, spike <=> g >= 0
                uq = t % UC
                nc.scalar.activation(
                    gc[:, (uq - n + 1) * F:(uq + 1) * F],
                    uc[:, (uq - n + 1) * F:(uq + 1) * F],
                    Sign, bias=0.0, scale=1.0,
                )

            for (t0, ch, xt) in x_tiles:
                for k in range(ch):
                    t = t0 + k
                    if t % UC == 0:
                        uc = up.tile([P, UC * F], mybir.dt.float32, tag="uchunk")
                        gc = gp.tile([P, UC * F], mybir.dt.float16, tag="gchunk")
                    cur = uc[:, (t % UC) * F:(t % UC + 1) * F]
                    nc.vector._custom_dve(
                        lif_op, out=cur, in0=prev,
                        in1=xt[:, k * F:(k + 1) * F], s0=0.5,
                    )
                    prev = cur
                    if t >= T - TAIL:
                        # final single-step groups: SWDGE cast-store the raw
                        # u slice (fp32 -> fp16, sign-faithful) — no Sign op,
                        # no PE round trip, minimal post-compute chain
                        m = GRP_OF[t][0]
                        nc.gpsimd.dma_start(
                            out=pk_ext[:, m * F:(m + 1) * F],
                            in_=uc[:, (t % UC) * F:(t % UC + 1) * F],
                        )
                        continue
                    if (t + 1) % CAST == 0:
                        sign_n = CAST
                    elif t == T - TAIL - 1:
                        sign_n = (t + 1) % CAST  # partial piece before tail
                    else:
                        sign_n = 0
                    if sign_n:
                        emit_signs(t, sign_n)
                        # balanced-ternary pack on the (otherwise idle) PE:
                        #   psum[m] = sum_{k<GL} 3^k * g_{GL*m+k}  (exact ints)
                        for tt in range(t - sign_n + 1, t + 1):
                            m, gs, gl = GRP_OF[tt]
                            gk = tt - gs
                            if gl == 1:
                                # single-step group: the Sign tile IS the
                                # packed value — skip the PE round trip
                                pk_tiles.append(
                                    (m, gc[:, (tt % UC) * F:(tt % UC + 1) * F])
                                )
                                continue
                            if gk == 0:
                                pk_ps = pp.tile([P, F], mybir.dt.float32,
                                                tag="packps")
                            last = gk == gl - 1
                            nc.tensor.matmul(
                                pk_ps[:],
                                w3[:, gk * P:(gk + 1) * P],
                                gc[:, (tt % UC) * F:(tt % UC + 1) * F],
                                start=(gk == 0), stop=last,
                            )
                            if last:
                                pk_sb = kp.tile([P, F], mybir.dt.float16,
                                                tag=f"pk{m}")
                                nc.scalar.activation(
                                    pk_sb[:], pk_ps[:], Copy,
                                    bias=0.0, scale=1.0,
                                )
                                pk_tiles.append((m, pk_sb[:]))

            # output stores: only 1.25 MiB total, a trickle next to the
            # 16 MiB load stream — no gating needed.  They ride the
            # otherwise-idle gpsimd (SWDGE) queue.
            for (m, pt) in pk_tiles:
                nc.gpsimd.dma_start(
                    out=pk_ext[:, m * F:(m + 1) * F], in_=pt,
                )

    nc.compile()
    _built["nc"] = nc
    return nc


def _install_ntff_hook() -> bool:
    """Provide antenv.axon_hooks (absent in this image) so that
    run_bass_kernel_spmd(trace=True) can capture NTFF profiles via the
    ctypes hook that trn_agent_boot already implements."""
    try:
        from antenv.axon_hooks import get_axon_ntff_profile_hook  # noqa: F401
        return True
    except ImportError:
        pass
    try:
        import sys
        import types
        import antenv
        from trn_agent_boot.trn_boot import _ntff_profile_via_ctypes

        hook = _ntff_profile_via_ctypes("/opt/axon/libaxon_pjrt.so")
        if hook is None:
            return False
        mod = types.ModuleType("antenv.axon_hooks")
        state = {"hook": hook}
        mod.get_axon_ntff_profile_hook = lambda: state["hook"]
        mod.set_axon_ntff_profile_hook = lambda h: state.__setitem__("hook", h)
        sys.modules["antenv.axon_hooks"] = mod
        antenv.axon_hooks = mod
        return True
    except Exception:
        return False


def kernel(x: np.ndarray) -> np.ndarray:
    import concourse.bass_utils as bass_utils

    nc = _build()

    x = np.asarray(x)
    assert x.shape == (T, B, N) and x.dtype == np.float32

    xs = x - np.float32(2.0)
    # 3^k * I weights for the PE spike packer (fp16 ints, exact up to 3^6)
    w3 = np.concatenate(
        [(3.0 ** k) * np.eye(P, dtype=np.float16) for k in range(GL)], axis=1
    ).astype(np.float16)
    in_maps = []
    for c in range(NCORES):
        # [T, BL*N] -> [T, P, F] -> [P, T, F] -> [P, T*F]  (partition-major)
        shard = (
            xs[:, c * BL:(c + 1) * BL, :]
            .reshape(T, P, F)
            .transpose(1, 0, 2)
            .reshape(P, T * F)
        )
        in_maps.append({"x": np.ascontiguousarray(shard), "w3": w3})

    trace = bool(int(os.environ.get("LIF_TRACE", "0")))
    if trace:
        trace = _install_ntff_hook()
        # artifact upload has no bucket in this container; neuter it
        bass_utils.upload_artifacts = lambda tmpdir: tmpdir

    try:
        res = bass_utils.run_bass_kernel_spmd(
            nc, in_maps, list(range(NCORES)), trace=trace
        )
    except Exception:
        if not trace:
            raise
        res = bass_utils.run_bass_kernel_spmd(
            nc, in_maps, list(range(NCORES)), trace=False
        )
    _built["last_result"] = res

    out = np.empty((T, B, N), np.float32)
    for c in range(NCORES):
        pk_f = np.asarray(res.results[c]["pk"]).reshape(P, NG, F)
        pk = pk_f.astype(np.int32)                     # exact small ints
        spikes = np.empty((T, P, F), np.bool_)
        # balanced-ternary decode: digit_k >= 0  <=>  sign(u) in {0,+1}
        for m, (gs, gl) in enumerate(GROUPS):
            if gl == 1:
                # raw fp16-cast u: spike <=> sign bit clear
                spikes[gs] = ~np.signbit(pk_f[:, m, :])
                continue
            q = pk[:, m, :] + (3 ** gl - 1) // 2
            for k in range(gl):
                spikes[gs + k] = (q // 3 ** k) % 3 >= 1
        out[:, c * BL:(c + 1) * BL, :] = (
            spikes.astype(np.float32).reshape(T, BL, N)
        )
    return out
